# revision 11
# baseline (speedup 1.0000x reference)
"""Trainium2 Bass kernel for the rate-1/2 K=7 convolutional encoder.

Math: with the shift-register trellis (to_nodes[s,u] = (u<<5)|(s>>1)) and a
GF(2)-affine output table, output bit j at time t is the XOR (parity) of a
fixed subset of the 7-bit sliding window (x_t, ..., x_{t-6}).  The time
recurrence disappears entirely: each output plane is a shifted-XOR tree over
the input bit matrix, fully parallel over batch and time.

Device pipeline per core (256 batch rows, batch-parallel across 8 cores):
  HBM f32 --(gpsimd casting DMA)--> SBUF bf16 `xb` (with zero left-pad)
  --(1-element-shifted copy `xb1`, so odd time shifts stay 4B-aligned and
     VectorE runs its 2x bf16 mode)-->
  --(VectorE logical_xor tree over shifted slices)--> parity planes p0, p1
  --(ScalarE strided copies, bf16 -> f32, interleaving the two planes)-->
  --(DMA)--> HBM f32 [256, 8192]

Host side only inspects/validates the 64x2 trellis tables to derive the tap
sets; if the tables do not have the expected structure, a numpy emulation of
the reference is used as a fallback.
"""

import sys

sys.path.insert(0, "/opt/trn_rl_repo")

import numpy as np

N_CORES = 8
B, K = 2048, 4096
B_CORE = B // N_CORES       # 256
MU = 6                      # encoder memory (64 states)
PAD = 8                     # zero left-pad elements (covers shifts up to 7)
CH = 1024                   # time chunk per compute step
N_CH = K // CH              # 4


def _extract_taps(to_nodes, out_sym):
    """If the tables are a shift-register trellis with an affine-linear output
    map, return (taps[2,7], const[2]); else None.

    Window bit d is x_{t-d}.  The combined lookup index is
    idx = 2*state + u with idx bit0 = x_t and idx bit (7-d) = x_{t-d}.
    """
    to_nodes = np.asarray(to_nodes)
    out_sym = np.asarray(out_sym)
    if to_nodes.shape != (64, 2) or out_sym.shape != (64, 2):
        return None
    s = np.arange(64)[:, None]
    u = np.arange(2)[None, :]
    if not np.array_equal(to_nodes, (u << (MU - 1)) | (s >> 1)):
        return None
    if out_sym.min() < 0 or out_sym.max() > 3:
        return None

    def bitpos(d):
        return 0 if d == 0 else 7 - d

    idx = np.arange(128)
    taps = np.zeros((2, 7), np.int64)
    const = np.zeros(2, np.int64)
    for j in range(2):
        f = (out_sym[idx >> 1, idx & 1] >> (1 - j)) & 1
        c = f[0]
        const[j] = c
        for d in range(7):
            taps[j, d] = f[1 << bitpos(d)] ^ c
        # verify affine-linearity on all 128 indices
        pred = np.full(128, c)
        for d in range(7):
            pred ^= taps[j, d] * ((idx >> bitpos(d)) & 1)
        if not np.array_equal(pred, f):
            return None
    return taps, const


def _numpy_fallback(inputs, to_nodes, out_sym):
    """Exact emulation of the reference for unstructured tables."""
    msg = np.asarray(inputs).astype(np.int32)
    b, k = msg.shape
    to_nodes = np.asarray(to_nodes)
    out_sym = np.asarray(out_sym)
    st = np.zeros(b, np.int32)
    syms = np.zeros((k, b), np.int32)
    for t in range(k):
        u = msg[:, t]
        syms[t] = out_sym[st, u]
        st = to_nodes[st, u]
    shifts = np.arange(1, -1, -1)
    bits = (syms[:, :, None] >> shifts) & 1
    return bits.transpose(1, 0, 2).reshape(b, 2 * k).astype(np.float32)


_CACHE = {}


def _build_bass(taps, const):
    import concourse.mybir as mybir
    import concourse.tile as tile
    from concourse import bacc

    f32 = mybir.dt.float32
    bf16 = mybir.dt.bfloat16
    XOR = mybir.AluOpType.logical_xor

    D = [sorted(d for d in range(7) if taps[j, d]) for j in range(2)]
    common = sorted(set(D[0]) & set(D[1]))
    rest = [sorted(set(D[j]) - set(common)) for j in range(2)]

    nc = bacc.Bacc(None, target_bir_lowering=False)
    x = nc.dram_tensor("x", [B_CORE, K], f32, kind="ExternalInput")
    y = nc.dram_tensor("y", [B_CORE, 2 * K], f32, kind="ExternalOutput")

    n_blk = B_CORE // 128

    with tile.TileContext(nc) as tc:
        with (
            tc.tile_pool(name="xb", bufs=2) as xpool,
            tc.tile_pool(name="tmp", bufs=3) as tpool,
            tc.tile_pool(name="ob", bufs=3) as opool,
        ):
            for blk in range(n_blk):
                rows = slice(blk * 128, (blk + 1) * 128)
                xb = xpool.tile([128, PAD + K], bf16, tag="xb")
                xb1 = xpool.tile([128, PAD + K], bf16, tag="xb1")
                nc.vector.memset(xb[:, 0:PAD], 0.0)
                nc.vector.memset(xb1[:, 0:PAD], 0.0)
                for q in range(N_CH):
                    # SWDGE DMA casts f32 -> bf16 in flight
                    nc.gpsimd.dma_start(
                        xb[:, PAD + q * CH:PAD + (q + 1) * CH],
                        x[rows, q * CH:(q + 1) * CH],
                    )
                    # xb1[e] = xb[e-1]: keeps odd shifts 4B-aligned
                    nc.gpsimd.dma_start(
                        xb1[:, PAD + q * CH:PAD + (q + 1) * CH],
                        xb[:, PAD + q * CH - 1:PAD + (q + 1) * CH - 1],
                    )

                def term_ap(d, q):
                    off = PAD + CH * q - d
                    if d % 2 == 0:
                        return xb[:, off:off + CH]
                    return xb1[:, off + 1:off + 1 + CH]

                def xor_reduce(aps, q, tag):
                    """Pairwise-xor a list of APs down to one AP."""
                    aps = list(aps)
                    i = 0
                    while len(aps) > 1:
                        nxt = []
                        for a in range(0, len(aps) - 1, 2):
                            t = tpool.tile([128, CH], bf16, tag=f"{tag}{i}{a}")
                            nc.vector.tensor_tensor(
                                t[:], aps[a], aps[a + 1], XOR
                            )
                            nxt.append(t[:])
                        if len(aps) % 2:
                            nxt.append(aps[-1])
                        aps = nxt
                        i += 1
                    return aps[0]

                for q in range(N_CH):
                    com_ap = None
                    if len(common) >= 1:
                        com_ap = xor_reduce(
                            [term_ap(d, q) for d in common], q, "c"
                        )
                    planes = []
                    for j in range(2):
                        aps = [term_ap(d, q) for d in rest[j]]
                        if com_ap is not None:
                            aps.append(com_ap)
                        planes.append(xor_reduce(aps, q, f"p{j}"))

                    ob = opool.tile([128, 2 * CH], f32, tag="ob")
                    for j in range(2):
                        # strided interleave + bf16->f32 cast on ScalarE;
                        # const[j]==1 folds in as out = 1 - in
                        scale = -1.0 if const[j] else 1.0
                        bias = 1.0 if const[j] else 0.0
                        nc.scalar.activation(
                            ob[:, j:2 * CH:2],
                            planes[j],
                            mybir.ActivationFunctionType.Copy,
                            bias=bias,
                            scale=scale,
                        )
                    nc.sync.dma_start(
                        y[rows, 2 * CH * q:2 * CH * (q + 1)], ob[:]
                    )
    nc.finalize()
    return nc


def kernel(inputs, to_nodes, out_sym):
    inputs = np.ascontiguousarray(np.asarray(inputs, dtype=np.float32))
    tc_ = _extract_taps(to_nodes, out_sym)
    if tc_ is None or any(len([d for d in range(7) if tc_[0][j, d]]) < 2 for j in range(2)):
        return _numpy_fallback(inputs, to_nodes, out_sym)
    taps, const = tc_

    from concourse.bass_utils import run_bass_kernel_spmd

    key = (taps.tobytes(), const.tobytes())
    if key not in _CACHE:
        _CACHE[key] = _build_bass(taps, const)
    nc = _CACHE[key]

    in_maps = [
        {"x": inputs[c * B_CORE:(c + 1) * B_CORE]} for c in range(N_CORES)
    ]
    res = run_bass_kernel_spmd(nc, in_maps, core_ids=list(range(N_CORES)))
    out = np.concatenate(
        [np.asarray(r["y"], dtype=np.float32) for r in res.results], axis=0
    )
    return out


# revision 13
# speedup vs baseline: 1.0012x; 1.0012x over previous
"""Trainium2 Bass kernel for the rate-1/2 K=7 convolutional encoder.

Math: with the shift-register trellis (to_nodes[s,u] = (u<<5)|(s>>1)) and a
GF(2)-affine output table, output bit j at time t is the XOR (parity) of a
fixed subset of the 7-bit sliding window (x_t, ..., x_{t-6}).  The time
recurrence disappears entirely: each output plane is a shifted-XOR tree over
the input bit matrix, fully parallel over batch and time.

Device pipeline per core (256 batch rows, batch-parallel across 8 cores):
  HBM f32 --(gpsimd casting DMA)--> SBUF bf16 `xb` (with zero left-pad)
  --(1-element-shifted copy `xb1`, so odd time shifts stay 4B-aligned and
     VectorE runs its 2x bf16 mode)-->
  --(VectorE logical_xor tree over shifted slices)--> parity planes p0, p1
  --(ScalarE strided copies, bf16 -> f32, interleaving the two planes)-->
  --(DMA)--> HBM f32 [256, 8192]

Host side only inspects/validates the 64x2 trellis tables to derive the tap
sets; if the tables do not have the expected structure, a numpy emulation of
the reference is used as a fallback.
"""

import sys

sys.path.insert(0, "/opt/trn_rl_repo")

import numpy as np

N_CORES = 8
B, K = 2048, 4096
B_CORE = B // N_CORES       # 256
MU = 6                      # encoder memory (64 states)
PAD = 8                     # zero left-pad elements (covers shifts up to 7)
CH = 1024                   # time chunk per compute step
N_CH = K // CH              # 4


def _extract_taps(to_nodes, out_sym):
    """If the tables are a shift-register trellis with an affine-linear output
    map, return (taps[2,7], const[2]); else None.

    Window bit d is x_{t-d}.  The combined lookup index is
    idx = 2*state + u with idx bit0 = x_t and idx bit (7-d) = x_{t-d}.
    """
    to_nodes = np.asarray(to_nodes)
    out_sym = np.asarray(out_sym)
    if to_nodes.shape != (64, 2) or out_sym.shape != (64, 2):
        return None
    s = np.arange(64)[:, None]
    u = np.arange(2)[None, :]
    if not np.array_equal(to_nodes, (u << (MU - 1)) | (s >> 1)):
        return None
    if out_sym.min() < 0 or out_sym.max() > 3:
        return None

    def bitpos(d):
        return 0 if d == 0 else 7 - d

    idx = np.arange(128)
    taps = np.zeros((2, 7), np.int64)
    const = np.zeros(2, np.int64)
    for j in range(2):
        f = (out_sym[idx >> 1, idx & 1] >> (1 - j)) & 1
        c = f[0]
        const[j] = c
        for d in range(7):
            taps[j, d] = f[1 << bitpos(d)] ^ c
        # verify affine-linearity on all 128 indices
        pred = np.full(128, c)
        for d in range(7):
            pred ^= taps[j, d] * ((idx >> bitpos(d)) & 1)
        if not np.array_equal(pred, f):
            return None
    return taps, const


def _numpy_fallback(inputs, to_nodes, out_sym):
    """Exact emulation of the reference for unstructured tables."""
    msg = np.asarray(inputs).astype(np.int32)
    b, k = msg.shape
    to_nodes = np.asarray(to_nodes)
    out_sym = np.asarray(out_sym)
    st = np.zeros(b, np.int32)
    syms = np.zeros((k, b), np.int32)
    for t in range(k):
        u = msg[:, t]
        syms[t] = out_sym[st, u]
        st = to_nodes[st, u]
    shifts = np.arange(1, -1, -1)
    bits = (syms[:, :, None] >> shifts) & 1
    return bits.transpose(1, 0, 2).reshape(b, 2 * k).astype(np.float32)


_CACHE = {}


def _build_bass(taps, const):
    import concourse.mybir as mybir
    import concourse.tile as tile
    from concourse import bacc

    f32 = mybir.dt.float32
    bf16 = mybir.dt.bfloat16
    XOR = mybir.AluOpType.logical_xor

    D = [sorted(d for d in range(7) if taps[j, d]) for j in range(2)]
    common = sorted(set(D[0]) & set(D[1]))
    rest = [sorted(set(D[j]) - set(common)) for j in range(2)]

    nc = bacc.Bacc(None, target_bir_lowering=False)
    x = nc.dram_tensor("x", [B_CORE, K], f32, kind="ExternalInput")
    y = nc.dram_tensor("y", [B_CORE, 2 * K], f32, kind="ExternalOutput")

    n_blk = B_CORE // 128

    with tile.TileContext(nc) as tc:
        with (
            tc.tile_pool(name="xb", bufs=2) as xpool,
            tc.tile_pool(name="tmp", bufs=3) as tpool,
            tc.tile_pool(name="ob", bufs=3) as opool,
        ):
            for blk in range(n_blk):
                rows = slice(blk * 128, (blk + 1) * 128)
                xb = xpool.tile([128, PAD + K], bf16, tag="xb")
                xb1 = xpool.tile([128, PAD + K], bf16, tag="xb1")
                nc.vector.memset(xb[:, 0:PAD], 0.0)
                nc.vector.memset(xb1[:, 0:PAD], 0.0)
                for q in range(N_CH):
                    # SWDGE DMA casts f32 -> bf16 in flight
                    nc.gpsimd.dma_start(
                        xb[:, PAD + q * CH:PAD + (q + 1) * CH],
                        x[rows, q * CH:(q + 1) * CH],
                    )
                    # xb1[e] = xb[e-1]: keeps odd shifts 4B-aligned.
                    # ScalarE copy, not DMA: the SDMA engines are the
                    # bottleneck and ScalarE has headroom.
                    nc.scalar.copy(
                        xb1[:, PAD + q * CH:PAD + (q + 1) * CH],
                        xb[:, PAD + q * CH - 1:PAD + (q + 1) * CH - 1],
                    )

                def term_ap(d, q):
                    off = PAD + CH * q - d
                    if d % 2 == 0:
                        return xb[:, off:off + CH]
                    return xb1[:, off + 1:off + 1 + CH]

                def xor_reduce(aps, q, tag):
                    """Pairwise-xor a list of APs down to one AP."""
                    aps = list(aps)
                    i = 0
                    while len(aps) > 1:
                        nxt = []
                        for a in range(0, len(aps) - 1, 2):
                            t = tpool.tile([128, CH], bf16, tag=f"{tag}{i}{a}")
                            nc.vector.tensor_tensor(
                                t[:], aps[a], aps[a + 1], XOR
                            )
                            nxt.append(t[:])
                        if len(aps) % 2:
                            nxt.append(aps[-1])
                        aps = nxt
                        i += 1
                    return aps[0]

                for q in range(N_CH):
                    com_ap = None
                    if len(common) >= 1:
                        com_ap = xor_reduce(
                            [term_ap(d, q) for d in common], q, "c"
                        )
                    planes = []
                    for j in range(2):
                        aps = [term_ap(d, q) for d in rest[j]]
                        if com_ap is not None:
                            aps.append(com_ap)
                        planes.append(xor_reduce(aps, q, f"p{j}"))

                    # staged in bf16; the store DMA casts to f32, halving
                    # the SBUF-side read bytes of the store
                    ob = opool.tile([128, 2 * CH], bf16, tag="ob")
                    for j in range(2):
                        # strided interleave on ScalarE;
                        # const[j]==1 folds in as out = 1 - in
                        scale = -1.0 if const[j] else 1.0
                        bias = 1.0 if const[j] else 0.0
                        nc.scalar.activation(
                            ob[:, j:2 * CH:2],
                            planes[j],
                            mybir.ActivationFunctionType.Copy,
                            bias=bias,
                            scale=scale,
                        )
                    nc.gpsimd.dma_start(
                        y[rows, 2 * CH * q:2 * CH * (q + 1)], ob[:]
                    )
    nc.finalize()
    return nc


def kernel(inputs, to_nodes, out_sym):
    inputs = np.ascontiguousarray(np.asarray(inputs, dtype=np.float32))
    tc_ = _extract_taps(to_nodes, out_sym)
    if tc_ is None or any(len([d for d in range(7) if tc_[0][j, d]]) < 2 for j in range(2)):
        return _numpy_fallback(inputs, to_nodes, out_sym)
    taps, const = tc_

    from concourse.bass_utils import run_bass_kernel_spmd

    key = (taps.tobytes(), const.tobytes())
    if key not in _CACHE:
        _CACHE[key] = _build_bass(taps, const)
    nc = _CACHE[key]

    in_maps = [
        {"x": inputs[c * B_CORE:(c + 1) * B_CORE]} for c in range(N_CORES)
    ]
    res = run_bass_kernel_spmd(nc, in_maps, core_ids=list(range(N_CORES)))
    out = np.concatenate(
        [np.asarray(r["y"], dtype=np.float32) for r in res.results], axis=0
    )
    return out
